# revision 13
# baseline (speedup 1.0000x reference)
"""Trainium2 Bass kernel for nn_ContrastLoss (smooth-histogram contrast loss).

Algorithm
---------
reference computes, per image:  hist[b] = sum_p w(x_p,b) / (S_p + 1e-8),
w = exp(-0.5*((x - c_b)/sigma)^2), c_b = b/255, sigma = 0.01, S_p = sum_b w,
followed by MSEs between the three histograms.

sigma is tiny, so hist is (up to fine quantization of x) a fixed linear map of
the fine count histogram of u = round(x * 4080) in [0, 4080]:
    hist[b] = sum_u cnt[u] * Phi[u, b]
The device only needs cnt[4096] per image — a pure counting problem.

Device kernel (SPMD over 8 cores, data-parallel over pixels):
  - per core/image, 32768 pixels in SBUF [128, 256]; u = round(4080 x) via the
    2^23 magic-add; split u = WL*hi + lo.
  - counting via PE outer products: for a column-group of pixels, one-hot(hi)
    and one-hot(lo) (batched broadcast is_equal on DVE/GPSIMD, fp8 output) are
    multiplied on the PE, accumulating the exact 2-D count table in f32 PSUM:
        cnt2[hi, lo] += onehot(hi)^T @ onehot(lo)
  - two throughput tricks on the PE: fp8 DoubleRow packs two 128-pixel k-tiles
    per weight load, and NG independent pixel groups ride block-diagonally in
    one matmul (cross-group quadrants of the PSUM table are garbage that the
    extraction step simply skips).  One matmul counts 2*NG*128 pixels.
Host: sum counts over the 8 cores (the all-reduce), apply the exact f64
cell-averaged Phi map, then the MSE.  Rel-err vs the f32 reference ~1.1e-4
(fine-grid quantization noise), far inside fp32 tolerance for this reduction.
"""

import os
import sys

import numpy as np

for _p in ("/opt/trn_rl_repo", "/root/.axon_site/_ro/trn_rl_repo"):
    if os.path.isdir(_p) and _p not in sys.path:
        sys.path.insert(0, _p)

import concourse.bass as bass  # noqa: E402
import concourse.tile as tile  # noqa: E402
from concourse import bacc, mybir  # noqa: E402
from concourse.bass_utils import run_bass_kernel_spmd, axon_active  # noqa: E402

N_CORES = 8
N_IMG = 3
IMG_PIX = 4 * 1 * 256 * 256          # 262144 pixels per image
SHARD = IMG_PIX // N_CORES           # 32768 pixels per core per image
P, T = 128, 256                      # on-chip pixel layout (SHARD = P*T)
NG = 2                               # pixel groups per matmul (block-diagonal)
WH = 128 // NG                       # hi one-hot width
WL = 4096 // WH                      # lo one-hot width
GRID = WH * WL                       # 4096 fine levels, u = WL*hi + lo
SCALE = 4080.0                       # u = round(x * 4080) in [0, 4080]
MAGIC = 8388608.0                    # 2**23: float32 round-to-nearest trick
SIGMA = 0.01
BINS = 256
TPM = 2 * NG                         # pixel-tiles (columns of T) per matmul
NMM = T // TPM                       # matmuls per image
CHUNK_MM = 8                         # matmuls per one-hot build instruction
GPS_R_TILES = 136                    # trailing R tiles per image built on GPSIMD

_CACHE = {}


def _build_program():
    nc = bacc.Bacc(
        "TRN2",
        target_bir_lowering=False,
        debug=not axon_active(),
        num_devices=N_CORES,
    )
    f32 = mybir.dt.float32
    fp8 = mybir.dt.float8e4
    A = mybir.AluOpType
    DR = mybir.MatmulPerfMode.DoubleRow

    x_d = nc.dram_tensor("x", [N_IMG, P, T], f32, kind="ExternalInput")
    iota_d = nc.dram_tensor("iota", [P, WL], f32, kind="ExternalInput")
    cnt_d = nc.dram_tensor("cnt", [N_IMG, WH, WL], f32, kind="ExternalOutput")

    with tile.TileContext(nc) as tc:
        with (
            tc.tile_pool(name="pool", bufs=3) as pool,
            tc.tile_pool(name="cpool", bufs=1) as cpool,
            tc.tile_pool(name="psum", bufs=1, space=bass.MemorySpace.PSUM) as pp,
        ):
            iota = cpool.tile([P, WL], f32, tag="iota")
            nc.sync.dma_start(iota[:], iota_d[:])

            for i in range(N_IMG):
                x = pool.tile([P, T], f32, tag="x")
                nc.sync.dma_start(x[:], x_d[i])

                # prep runs on the otherwise-idle ACT engine (Copy applies
                # in*scale + bias).  u = round(x*SCALE) via the 2^23 magic-add;
                # hi = round((u - (WL/2-.5))/WL) with a +8 shift keeping the
                # magic-add argument above 2^23 where f32 spacing is 1.
                CP = mybir.ActivationFunctionType.Copy
                t0 = pool.tile([P, T], f32, tag="t0")
                nc.scalar.activation(t0[:], x[:], CP, bias=MAGIC, scale=SCALE)
                u = pool.tile([P, T], f32, tag="u")
                nc.scalar.activation(u[:], t0[:], CP, bias=-MAGIC)
                t1 = pool.tile([P, T], f32, tag="t1")
                nc.scalar.activation(
                    t1[:], u[:], CP,
                    bias=-(WL / 2.0 - 0.5) / WL, scale=1.0 / WL,
                )
                t2 = pool.tile([P, T], f32, tag="t2")
                nc.scalar.activation(t2[:], t1[:], CP, bias=MAGIC + 8.0)
                hi = pool.tile([P, T], f32, tag="hi")
                nc.scalar.activation(hi[:], t2[:], CP, bias=-(MAGIC + 8.0))
                lo = pool.tile([P, T], f32, tag="lo")
                nc.vector.scalar_tensor_tensor(
                    lo[:], hi[:], -float(WL), u[:], A.mult, A.add
                )

                # one-hot buffers: tile-column tau gets onehot(hi[:,tau]) [WH]
                # and onehot(lo[:,tau]) [WL].  DVE does batched broadcast
                # is_equal; GPSIMD (which only supports the tensor_scalar
                # form) takes a contiguous block of R tiles per image.
                Lb = pool.tile([P, T, WH], fp8, tag="Lb")
                Rb = pool.tile([P, T, WL], fp8, tag="Rb")
                gps_start = T - GPS_R_TILES
                for c0 in range(0, T, CHUNK_MM * TPM):
                    c1 = min(c0 + CHUNK_MM * TPM, T)
                    n = c1 - c0
                    nc.vector.tensor_tensor(
                        Lb[:, c0:c1, :],
                        iota[:, None, 0:WH].broadcast_to([P, n, WH]),
                        hi[:, c0:c1, None].broadcast_to([P, n, WH]),
                        A.is_equal,
                    )
                    dv1 = min(c1, max(c0, gps_start))
                    if dv1 > c0:
                        nc.vector.tensor_tensor(
                            Rb[:, c0:dv1, :],
                            iota[:, None, 0:WL].broadcast_to([P, dv1 - c0, WL]),
                            lo[:, c0:dv1, None].broadcast_to([P, dv1 - c0, WL]),
                            A.is_equal,
                        )
                    for c in range(max(c0, gps_start), c1):
                        nc.gpsimd.tensor_scalar(
                            Rb[:, c, :], iota[:, 0:WL], lo[:, c : c + 1],
                            None, A.is_equal,
                        )

                ps = pp.tile([NG * WH, NG * WL], f32, tag="ps")
                for t in range(NMM):
                    lhsT = Lb[:, t * TPM : (t + 1) * TPM, :].rearrange(
                        "p (j g) w -> p j (g w)", j=2
                    )
                    rhs = Rb[:, t * TPM : (t + 1) * TPM, :].rearrange(
                        "p (j g) w -> p j (g w)", j=2
                    )
                    nc.tensor.matmul(
                        ps[:],
                        lhsT,
                        rhs,
                        start=(t == 0),
                        stop=(t == NMM - 1),
                        perf_mode=DR,
                    )

                # extract the valid diagonal blocks: cnt2 = sum_g ps[gWH:, gWL:]
                res = pool.tile([WH, WL], f32, tag="res")
                nc.vector.tensor_copy(res[:], ps[0:WH, 0:WL])
                for g in range(1, NG):
                    nc.vector.tensor_tensor(
                        res[:],
                        res[:],
                        ps[g * WH : (g + 1) * WH, g * WL : (g + 1) * WL],
                        A.add,
                    )
                nc.sync.dma_start(cnt_d[i], res[:])

    nc.compile()
    return nc


def _phi():
    """f64 [GRID, BINS] map: cell-averaged smooth-histogram contribution."""
    b = np.arange(BINS, dtype=np.float64)
    step = SCALE / 255.0
    u_grid = np.arange(GRID, dtype=np.float64)
    nsub = 17
    offs = np.linspace(-0.5, 0.5, nsub)
    wts = np.ones(nsub)
    wts[1:-1:2], wts[2:-1:2] = 4.0, 2.0
    wts /= wts.sum()
    phi = np.zeros((GRID, BINS))
    for o, ws in zip(offs, wts):
        diff = ((u_grid + o)[:, None] - step * b[None, :]) / SCALE
        w = np.exp(-0.5 * (diff / SIGMA) ** 2)
        phi += ws * (w / (w.sum(axis=1, keepdims=True) + 1e-8))
    return phi


def _iota_np():
    return np.broadcast_to(np.arange(WL, dtype=np.float32)[None, :], (P, WL)).copy()


def _get_state():
    if "nc" not in _CACHE:
        _CACHE["nc"] = _build_program()
        _CACHE["phi"] = _phi()
        _CACHE["iota"] = _iota_np()
    return _CACHE["nc"], _CACHE["phi"], _CACHE["iota"]


def _run_device(images, trace=False):
    """images: [3, IMG_PIX] f32 -> (results, counts [3, GRID] f64)."""
    nc, phi, iota = _get_state()
    in_maps = []
    for k in range(N_CORES):
        shard = images[:, k * SHARD : (k + 1) * SHARD].reshape(N_IMG, P, T)
        in_maps.append({"x": np.ascontiguousarray(shard), "iota": iota})
    res = run_bass_kernel_spmd(nc, in_maps, list(range(N_CORES)), trace=trace)
    cnt = np.zeros((N_IMG, GRID), dtype=np.float64)
    for k in range(N_CORES):
        cnt += res.results[k]["cnt"].astype(np.float64).reshape(N_IMG, GRID)
    return res, cnt


def kernel(fused_image, ir_image, visible_gray):
    imgs = np.stack(
        [
            np.asarray(fused_image, dtype=np.float32).reshape(-1),
            np.asarray(ir_image, dtype=np.float32).reshape(-1),
            np.asarray(visible_gray, dtype=np.float32).reshape(-1),
        ]
    )
    _, cnt = _run_device(imgs)
    _, phi, _ = _get_state()
    hists = cnt @ phi  # [3, 256] f64
    hf, hi_, hv = hists
    loss_ir = np.mean((hf - hi_) ** 2)
    loss_vis = np.mean((hf - hv) ** 2)
    return np.float32(0.5 * loss_ir + 0.5 * loss_vis)


# revision 15
# speedup vs baseline: 1.0031x; 1.0031x over previous
"""Trainium2 Bass kernel for nn_ContrastLoss (smooth-histogram contrast loss).

Algorithm
---------
reference computes, per image:  hist[b] = sum_p w(x_p,b) / (S_p + 1e-8),
w = exp(-0.5*((x - c_b)/sigma)^2), c_b = b/255, sigma = 0.01, S_p = sum_b w,
followed by MSEs between the three histograms.

sigma is tiny, so hist is (up to fine quantization of x) a fixed linear map of
the fine count histogram of u = round(x * 4080) in [0, 4080]:
    hist[b] = sum_u cnt[u] * Phi[u, b]
The device only needs cnt[4096] per image — a pure counting problem.

Device kernel (SPMD over 8 cores, data-parallel over pixels):
  - per core/image, 32768 pixels in SBUF [128, 256]; u = round(4080 x) via the
    2^23 magic-add; split u = WL*hi + lo.
  - counting via PE outer products: for a column-group of pixels, one-hot(hi)
    and one-hot(lo) (batched broadcast is_equal on DVE/GPSIMD, fp8 output) are
    multiplied on the PE, accumulating the exact 2-D count table in f32 PSUM:
        cnt2[hi, lo] += onehot(hi)^T @ onehot(lo)
  - two throughput tricks on the PE: fp8 DoubleRow packs two 128-pixel k-tiles
    per weight load, and NG independent pixel groups ride block-diagonally in
    one matmul (cross-group quadrants of the PSUM table are garbage that the
    extraction step simply skips).  One matmul counts 2*NG*128 pixels.
Host: sum counts over the 8 cores (the all-reduce), apply the exact f64
cell-averaged Phi map, then the MSE.  Rel-err vs the f32 reference ~1.1e-4
(fine-grid quantization noise), far inside fp32 tolerance for this reduction.
"""

import os
import sys

import numpy as np

for _p in ("/opt/trn_rl_repo", "/root/.axon_site/_ro/trn_rl_repo"):
    if os.path.isdir(_p) and _p not in sys.path:
        sys.path.insert(0, _p)

import concourse.bass as bass  # noqa: E402
import concourse.tile as tile  # noqa: E402
from concourse import bacc, mybir  # noqa: E402
from concourse.bass_utils import run_bass_kernel_spmd, axon_active  # noqa: E402

N_CORES = 8
N_IMG = 3
IMG_PIX = 4 * 1 * 256 * 256          # 262144 pixels per image
SHARD = IMG_PIX // N_CORES           # 32768 pixels per core per image
P, T = 128, 256                      # on-chip pixel layout (SHARD = P*T)
NG = 2                               # pixel groups per matmul (block-diagonal)
WH = 128 // NG                       # hi one-hot width
WL = 4096 // WH                      # lo one-hot width
GRID = WH * WL                       # 4096 fine levels, u = WL*hi + lo
SCALE = 4080.0                       # u = round(x * 4080) in [0, 4080]
MAGIC = 8388608.0                    # 2**23: float32 round-to-nearest trick
SIGMA = 0.01
BINS = 256
TPM = 2 * NG                         # pixel-tiles (columns of T) per matmul
NMM = T // TPM                       # matmuls per image
CHUNK_MM = 8                         # matmuls per one-hot build instruction
GPS_R_TILES = 136                    # trailing R tiles per image built on GPSIMD

_CACHE = {}


def _build_program():
    nc = bacc.Bacc(
        "TRN2",
        target_bir_lowering=False,
        debug=not axon_active(),
        num_devices=N_CORES,
    )
    f32 = mybir.dt.float32
    fp8 = mybir.dt.float8e4
    A = mybir.AluOpType
    DR = mybir.MatmulPerfMode.DoubleRow

    x_d = nc.dram_tensor("x", [N_IMG, P, T], f32, kind="ExternalInput")
    iota_d = nc.dram_tensor("iota", [P, WL], f32, kind="ExternalInput")
    cnt_d = nc.dram_tensor("cnt", [N_IMG, WH, WL], f32, kind="ExternalOutput")

    with tile.TileContext(nc) as tc:
        with (
            tc.tile_pool(name="pool", bufs=3) as pool,
            tc.tile_pool(name="cpool", bufs=1) as cpool,
            tc.tile_pool(name="psum", bufs=1, space=bass.MemorySpace.PSUM) as pp,
        ):
            iota = cpool.tile([P, WL], f32, tag="iota")
            nc.sync.dma_start(iota[:], iota_d[:])

            for i in range(N_IMG):
                x = pool.tile([P, T], f32, tag="x")
                nc.sync.dma_start(x[:], x_d[i])

                # prep runs on the otherwise-idle ACT engine (Copy applies
                # in*scale + bias).  u = round(x*SCALE) via the 2^23 magic-add;
                # hi = round((u - (WL/2-.5))/WL) with a +8 shift keeping the
                # magic-add argument above 2^23 where f32 spacing is 1.
                CP = mybir.ActivationFunctionType.Copy
                t0 = pool.tile([P, T], f32, tag="t0")
                nc.scalar.activation(t0[:], x[:], CP, bias=MAGIC, scale=SCALE)
                u = pool.tile([P, T], f32, tag="u")
                nc.scalar.activation(u[:], t0[:], CP, bias=-MAGIC)
                t1 = pool.tile([P, T], f32, tag="t1")
                nc.scalar.activation(
                    t1[:], u[:], CP,
                    bias=-(WL / 2.0 - 0.5) / WL, scale=1.0 / WL,
                )
                t2 = pool.tile([P, T], f32, tag="t2")
                nc.scalar.activation(t2[:], t1[:], CP, bias=MAGIC + 8.0)
                hi = pool.tile([P, T], f32, tag="hi")
                nc.scalar.activation(hi[:], t2[:], CP, bias=-(MAGIC + 8.0))
                lo = pool.tile([P, T], f32, tag="lo")
                nc.vector.scalar_tensor_tensor(
                    lo[:], hi[:], -float(WL), u[:], A.mult, A.add
                )

                # one-hot buffers: tile-column tau gets onehot(hi[:,tau]) [WH]
                # and onehot(lo[:,tau]) [WL].  DVE does batched broadcast
                # is_equal; GPSIMD (which only supports the tensor_scalar
                # form) takes a contiguous block of R tiles per image.
                Lb = pool.tile([P, T, WH], fp8, tag="Lb")
                Rb = pool.tile([P, T, WL], fp8, tag="Rb")
                gps_start = T - GPS_R_TILES
                for c0 in range(0, T, CHUNK_MM * TPM):
                    c1 = min(c0 + CHUNK_MM * TPM, T)
                    n = c1 - c0
                    nc.vector.tensor_tensor(
                        Lb[:, c0:c1, :],
                        iota[:, None, 0:WH].broadcast_to([P, n, WH]),
                        hi[:, c0:c1, None].broadcast_to([P, n, WH]),
                        A.is_equal,
                    )
                    dv1 = min(c1, max(c0, gps_start))
                    if dv1 > c0:
                        nc.vector.tensor_tensor(
                            Rb[:, c0:dv1, :],
                            iota[:, None, 0:WL].broadcast_to([P, dv1 - c0, WL]),
                            lo[:, c0:dv1, None].broadcast_to([P, dv1 - c0, WL]),
                            A.is_equal,
                        )
                    for c in range(max(c0, gps_start), c1):
                        nc.gpsimd.tensor_scalar(
                            Rb[:, c, :], iota[:, 0:WL], lo[:, c : c + 1],
                            None, A.is_equal,
                        )

                ps = pp.tile([NG * WH, NG * WL], f32, tag="ps")
                for t in range(NMM):
                    lhsT = Lb[:, t * TPM : (t + 1) * TPM, :].rearrange(
                        "p (j g) w -> p j (g w)", j=2
                    )
                    rhs = Rb[:, t * TPM : (t + 1) * TPM, :].rearrange(
                        "p (j g) w -> p j (g w)", j=2
                    )
                    nc.tensor.matmul(
                        ps[:],
                        lhsT,
                        rhs,
                        start=(t == 0),
                        stop=(t == NMM - 1),
                        perf_mode=DR,
                    )

                # extract the valid diagonal blocks: cnt2 = sum_g ps[gWH:, gWL:]
                res = pool.tile([WH, WL], f32, tag="res")
                nc.scalar.activation(res[:], ps[0:WH, 0:WL], CP, bias=0.0)
                for g in range(1, NG):
                    nc.vector.tensor_tensor(
                        res[:],
                        res[:],
                        ps[g * WH : (g + 1) * WH, g * WL : (g + 1) * WL],
                        A.add,
                    )
                nc.sync.dma_start(cnt_d[i], res[:])

    nc.compile()
    return nc


def _phi():
    """f64 [GRID, BINS] map: cell-averaged smooth-histogram contribution."""
    b = np.arange(BINS, dtype=np.float64)
    step = SCALE / 255.0
    u_grid = np.arange(GRID, dtype=np.float64)
    nsub = 17
    offs = np.linspace(-0.5, 0.5, nsub)
    wts = np.ones(nsub)
    wts[1:-1:2], wts[2:-1:2] = 4.0, 2.0
    wts /= wts.sum()
    phi = np.zeros((GRID, BINS))
    for o, ws in zip(offs, wts):
        diff = ((u_grid + o)[:, None] - step * b[None, :]) / SCALE
        w = np.exp(-0.5 * (diff / SIGMA) ** 2)
        phi += ws * (w / (w.sum(axis=1, keepdims=True) + 1e-8))
    return phi


def _iota_np():
    return np.broadcast_to(np.arange(WL, dtype=np.float32)[None, :], (P, WL)).copy()


def _get_state():
    if "nc" not in _CACHE:
        _CACHE["nc"] = _build_program()
        _CACHE["phi"] = _phi()
        _CACHE["iota"] = _iota_np()
    return _CACHE["nc"], _CACHE["phi"], _CACHE["iota"]


def _run_device(images, trace=False):
    """images: [3, IMG_PIX] f32 -> (results, counts [3, GRID] f64)."""
    nc, phi, iota = _get_state()
    in_maps = []
    for k in range(N_CORES):
        shard = images[:, k * SHARD : (k + 1) * SHARD].reshape(N_IMG, P, T)
        in_maps.append({"x": np.ascontiguousarray(shard), "iota": iota})
    res = run_bass_kernel_spmd(nc, in_maps, list(range(N_CORES)), trace=trace)
    cnt = np.zeros((N_IMG, GRID), dtype=np.float64)
    for k in range(N_CORES):
        cnt += res.results[k]["cnt"].astype(np.float64).reshape(N_IMG, GRID)
    return res, cnt


def kernel(fused_image, ir_image, visible_gray):
    imgs = np.stack(
        [
            np.asarray(fused_image, dtype=np.float32).reshape(-1),
            np.asarray(ir_image, dtype=np.float32).reshape(-1),
            np.asarray(visible_gray, dtype=np.float32).reshape(-1),
        ]
    )
    _, cnt = _run_device(imgs)
    _, phi, _ = _get_state()
    hists = cnt @ phi  # [3, 256] f64
    hf, hi_, hv = hists
    loss_ir = np.mean((hf - hi_) ** 2)
    loss_vis = np.mean((hf - hv) ** 2)
    return np.array(0.5 * loss_ir + 0.5 * loss_vis, dtype=np.float32)
